# revision 4
# baseline (speedup 1.0000x reference)
"""Trainium2 Bass kernel for nn_EquivariantScalar (segment_reduce), v2.

Restructured from the 401us baseline around measured engine busy times
(ACT 307us / PE 252us / GP 169us / DVE 157us on the baseline):
  - v-matmul outputs go to 3-bank PSUM "triples" [128,3,512] so each
    evacuation is ONE wide op instead of three (saves per-op overhead).
  - ACT keeps only its monopolies (Square-from-PSUM, Silu, Sqrt) plus
    block-0's triple square; block-1's square is split: component 0 on
    ACT, components 1-2 via DVE fp32->fp16 copy + DVE fp16 pair square.
  - The 3 gating multiplies fuse into ONE DVE broadcast multiply
    (ss broadcast over the component axis with a stride-0 AP).
  - Component sums are pair-batched [128,1024]: block-0 on DVE,
    block-1 on GpSimd (GpSimd cannot read PSUM; fp16 SBUF only).
  - Squares/copies live in every ACT table set, so only Silu/Sqrt ops
    are phase-fenced (2 table loads per group).
  - Modulo-scheduled emission: each k-step issues stages of DIFFERENT
    tiles (z/zt and v2b of tile k-1, h1/silu/tail1 of tile k, pre of
    tile k in the next group) so the in-order engine queues never sit
    on a not-yet-ready instruction.
  - PSUM: 2 triple slots (6 banks) + 2 H banks. pss reuses the dead
    ph1a bank and the z matmuls reuse the dead ph1b bank (a start=True
    matmul resets the whole bank, so only dead banks are reused).
    Pooling accumulates per group into a rotating H slot, folded into
    an SBUF ping-pong accumulator by one DVE add per group.
  - v and the one-hot batch mask ship as fp8e4 (half the DMA bytes;
    one-hot is exact, v's quantization is attenuated by the next
    matmul); v-group DMA issues two groups ahead.
"""
import os
import sys
import types

import numpy as np

NA = 200000
B = 256
NF = 128
NCORES = 8
SH = NA // NCORES          # 25000 atoms per core
T = 512                    # atoms per tile
NT = 49                    # tiles per core
NAs = NT * T               # 25088 padded atoms per core
G = 7                      # tiles per ACT-phase group
NG = NT // G               # 7 groups
GT = G * T                 # atoms per group
EPS = 1e-8

_prog_cache = {}


def _install_compat_patches():
    """Patches for this container: split multi-wait instructions (this
    walrus caps non-EventSemaphore instructions at ONE sync wait)."""
    import bass_rust
    from concourse import tile
    from concourse.vector_clock import ScopedClock

    if getattr(tile.TileContext, "_wait_split_patched", False):
        return

    def _patched_drain_and_barrier(self, tick_clock, wait_clock):
        nc = self.nc
        drain_inst = nc.sync.drain()
        wait_clock.add_sem_waits(
            drain_inst.ins, ScopedClock({None: tick_clock.global_clock})
        )
        si = drain_inst.ins.sync_info
        if si is not None and len(si.on_wait) > 1:
            waits = list(si.on_wait)
            si.on_wait = waits[:1]
            for w in waits[1:]:
                n = nc.sync.nop(nofuse=True, hint="tail_drain_wait_split")
                n.ins.sync_info = bass_rust.SyncInfo(on_wait=[w], on_update=[])
        nc.all_engine_barrier()
        assert self.sems is not None
        popped = nc._tile_sem_poison_stack.pop()
        assert popped is self._sem_poison
        nc.clear_and_free_semaphores(list(self.sems.allocated().values()))
        nc.all_engine_barrier()

    tile.TileContext._drain_and_barrier = _patched_drain_and_barrier
    tile.TileContext._wait_split_patched = True


def _legalize_waits(nc):
    if os.environ.get("SKIP_LEGALIZE"):
        return
    """Hoist extra sync waits (beyond the per-instruction HW capacity)
    onto fresh single-wait NoOps inserted just before, same engine."""
    import bass_rust
    import concourse.mybir as mybir

    counter = [0]
    for fn in nc.m.functions:
        for bb in fn.blocks:
            out = []
            changed = False
            for inst in bb.instructions:
                si = getattr(inst, "sync_info", None)
                waits = list(si.on_wait) if si is not None else []
                cap = 2 if isinstance(inst, mybir.InstEventSemaphore) else 1
                if len(waits) > cap:
                    si.on_wait = waits[:cap]
                    for w in waits[cap:]:
                        counter[0] += 1
                        n = mybir.InstNoOp(name=f"waitsplit-{counter[0]}")
                        n.engine = inst.engine
                        n.sync_info = bass_rust.SyncInfo(on_wait=[w], on_update=[])
                        out.append(n)
                    changed = True
                out.append(inst)
            if changed:
                bb.instructions = out


def _maybe_install_trace_shim():
    """Optional: NTFF profiling under axon needs antenv.axon_hooks."""
    try:
        import antenv

        if "antenv.axon_hooks" in sys.modules:
            return
        mod = types.ModuleType("antenv.axon_hooks")
        hook = [None]
        mod.set_axon_ntff_profile_hook = lambda h: hook.__setitem__(0, h)
        mod.get_axon_ntff_profile_hook = lambda: hook[0]
        sys.modules["antenv.axon_hooks"] = mod
        antenv.axon_hooks = mod
        from trn_agent_boot.trn_boot import _ntff_profile_via_ctypes

        mod.set_axon_ntff_profile_hook(
            _ntff_profile_via_ctypes("/opt/axon/libaxon_pjrt.so")
        )
    except Exception:
        pass


def _build_program():
    import concourse.bass as bass
    import concourse.mybir as mybir
    from concourse.tile import TileContext
    from concourse.tile_rust import add_dep_helper
    from concourse.alu_op_type import AluOpType

    F = mybir.dt.float32
    F16 = mybir.dt.float16
    F8 = mybir.dt.float8e4
    AF = mybir.ActivationFunctionType

    nc = bass.Bass()

    vT = nc.dram_tensor("vT", [NF, 3, NAs], F8, kind="ExternalInput")
    sT = nc.dram_tensor("sT", [NF, NAs], F16, kind="ExternalInput")
    bT = nc.dram_tensor("bT", [NAs, B], F8, kind="ExternalInput")

    wnames = [
        "wv1_0", "wv2_0", "wh1s_0", "wh1n_0", "wss_0",
        "wv2_1", "wh1sg_1", "wh1n_1",
    ]
    wdram = {n: nc.dram_tensor(n, [NF, NF], F16, kind="ExternalInput") for n in wnames}
    weff_d = nc.dram_tensor("weff", [NF, 1], F16, kind="ExternalInput")
    bias_names = ["b1_0", "bss_0", "b1_1", "beff", "eps"]
    bdram = {n: nc.dram_tensor(n, [NF, 1], F, kind="ExternalInput") for n in bias_names}

    y = nc.dram_tensor("y", [1, B], F, kind="ExternalOutput")

    from contextlib import ExitStack

    with TileContext(nc) as tc:
        with ExitStack() as _stk:
            _p = lambda **kw: _stk.enter_context(tc.tile_pool(**kw))
            wp = _p(name="wp", bufs=1)
            vin_p = _p(name="vin", bufs=3)       # [128,3,GT] f8 = 10.7KB/p
            sin_p = _p(name="sin", bufs=2)       # [128,GT]  f16 = 7KB/p
            bin_p = _p(name="bin", bufs=G + 1)   # [128,4,B] f16 = 2KB/p
            sq0_p = _p(name="sq0", bufs=2)       # [128,3,2T] f16 pair 6KB/p
            sq1_p = _p(name="sq1", bufs=2)       # [128,3,2T] f16 pair
            cp12_p = _p(name="cp12", bufs=2)     # [128,2,2T] f16 pair 4KB/p
            n01_p = _p(name="n01", bufs=2)       # [128,2T] f16 pair scratch
            n2a_p = _p(name="n2a", bufs=2)       # [128,GT] f16 group
            n2b_p = _p(name="n2b", bufs=2)
            nrm_p = _p(name="nrm", bufs=1)
            g2a_p = _p(name="g2a", bufs=2 * G + 2)
            g2b_p = _p(name="g2b", bufs=G + 2)
            ss_p = _p(name="ssb", bufs=4)
            vn_p = _p(name="vnw", bufs=3)        # [128,3,T] f16
            zz_p = _p(name="zz", bufs=G + 1)
            yo_p = _p(name="yo", bufs=1)
            psT = _p(name="psT", bufs=2, space="PSUM")   # [128,3,T] f32 x2
            psH = _p(name="psH", bufs=2, space="PSUM")   # [128,T] f32 x2
            w = {}
            for n in wnames:
                w[n] = wp.tile([NF, NF], F16, tag=n, name=n)
                nc.sync.dma_start(out=w[n][:], in_=wdram[n][:])
            weff = wp.tile([NF, 1], F16, tag="weff")
            nc.sync.dma_start(out=weff[:], in_=weff_d[:])
            bias = {}
            for n in bias_names:
                bias[n] = wp.tile([NF, 1], F, tag=n, name=n)
                nc.sync.dma_start(out=bias[n][:], in_=bdram[n][:])

            # Silu and Sqrt live in different ACT table sets; fence only
            # those two op families into per-group phases (2 table loads
            # per group). Square/Copy are in every set and float freely.
            _fence = [None]
            _phase = [[]]

            def _pin(inst):
                if _fence[0] is not None:
                    add_dep_helper(
                        inst.ins, _fence[0].ins, sync=False,
                        reason="ACT table-set phase order",
                    )
                _phase[0].append(inst)
                return inst

            def _end_phase():
                if not _phase[0]:
                    return
                fence = nc.scalar.nop(nofuse=True, hint="act_phase_fence")
                for i in _phase[0]:
                    add_dep_helper(
                        fence.ins, i.ins, sync=False,
                        reason="ACT table-set phase fence",
                    )
                _fence[0] = fence
                _phase[0] = []

            # y accumulates per group in a rotating H-slot (PSUM), then one
            # DVE add per group folds it into an SBUF ping-pong accumulator.
            ysb = [yo_p.tile([1, B], F, tag=f"ysb{i}", name=f"ysb{i}")
                   for i in range(2)]
            nc.gpsimd.memset(ysb[0][:], 0.0)

            S = {}            # per-tile state
            N2A, N2B = {}, {}  # per-group n^2 accumulators
            # tile pairs within a group: (0,1),(2,3),(4,5),(6,)
            PAIRS = [(0, 1), (2, 3), (4, 5), (6,)]

            def pair_of(ti):
                return PAIRS[min(ti // 2, 3)]

            def stage_pre(t):
                """v2a matmuls into a PSUM triple; one ACT square; on
                pair completion, DVE component sums into N2A."""
                g, ti = divmod(t, G)
                st = S[t]
                vt = st["vg"][:, :, ti * T: (ti + 1) * T]
                Ta = psT.tile([128, 3, T], F, tag="tri", name=f"v2a{t}")
                for c in range(3):
                    nc.tensor.matmul(Ta[:, c, :], w["wv2_0"][:], vt[:, c, :],
                                     start=True, stop=True)
                pr = pair_of(ti)
                pw = len(pr) * T
                if ti == pr[0]:
                    S[t]["sq0"] = sq0_p.tile([128, 3, 2 * T], F16, tag="sq0",
                                             name=f"sq0_{t}")
                else:
                    S[t]["sq0"] = S[t - 1]["sq0"]
                sq0 = S[t]["sq0"]
                half = (ti - pr[0]) * T
                nc.scalar.activation(sq0[:, :, half: half + T], Ta[:],
                                     AF.Square, bias=bias["eps"][:])
                if ti == 0:
                    N2A[g] = n2a_p.tile([128, GT], F16, tag="n2a",
                                        name=f"n2a_g{g}")
                if ti == pr[-1]:
                    a0 = pr[0] * T
                    n01 = n01_p.tile([128, 2 * T], F16, tag="n01a",
                                     name=f"n01a{t}")
                    nc.vector.tensor_add(n01[:, :pw], sq0[:, 0, :pw],
                                         sq0[:, 1, :pw])
                    nc.vector.tensor_add(N2A[g][:, a0: a0 + pw],
                                         n01[:, :pw], sq0[:, 2, :pw])

            def stage_tail1(t):
                """pss into the dead ph1a bank; ss; v1 matmuls; gating
                multiply (one bc op)."""
                st = S[t]
                g, ti = divmod(t, G)
                vt = st["vg"][:, :, ti * T: (ti + 1) * T]
                pss = st["ph1a"]
                nc.tensor.matmul(pss[:], w["wss_0"][:], st["g2a"][:],
                                 start=True, stop=True, skip_group_check=True)
                ss = ss_p.tile([128, T], F16, tag="ss", name=f"ss{t}")
                nc.vector.tensor_scalar_add(ss[:], pss[:], bias["bss_0"][:])
                Tb = psT.tile([128, 3, T], F, tag="tri", name=f"v1{t}")
                for c in range(3):
                    nc.tensor.matmul(Tb[:, c, :], w["wv1_0"][:], vt[:, c, :],
                                     start=True, stop=True)
                vnew = vn_p.tile([128, 3, T], F16, tag="vn", name=f"vn{t}")
                ss_b = ss[:].unsqueeze(1).broadcast_to((128, 3, T))
                nc.vector.tensor_tensor(out=vnew[:], in0=Tb[:], in1=ss_b,
                                        op=AluOpType.mult)
                S[t]["vnew"] = vnew
                del S[t]["ph1a"]

            def stage_tail2(t):
                """v2b matmuls; block-1 square split ACT/DVE; GP sums."""
                st = S[t]
                g, ti = divmod(t, G)
                Tc = psT.tile([128, 3, T], F, tag="tri", name=f"v2b{t}")
                for c in range(3):
                    nc.tensor.matmul(Tc[:, c, :], w["wv2_1"][:],
                                     st["vnew"][:, c, :],
                                     start=True, stop=True)
                del S[t]["vnew"]
                pr = pair_of(ti)
                pw = len(pr) * T
                if ti == pr[0]:
                    S[t]["sq1"] = sq1_p.tile([128, 3, 2 * T], F16, tag="sq1",
                                             name=f"sq1_{t}")
                    S[t]["cp12"] = cp12_p.tile([128, 2, 2 * T], F16,
                                               tag="cp12", name=f"cp12_{t}")
                else:
                    S[t]["sq1"] = S[t - 1]["sq1"]
                    S[t]["cp12"] = S[t - 1]["cp12"]
                sq1, cp12 = S[t]["sq1"], S[t]["cp12"]
                half = (ti - pr[0]) * T
                nc.vector.tensor_copy(cp12[:, :, half: half + T], Tc[:, 1:3, :])
                nc.scalar.activation(sq1[:, 0, half: half + T], Tc[:, 0, :],
                                     AF.Square)
                if ti == pr[-1]:
                    a0 = pr[0] * T
                    nc.vector.tensor_tensor(
                        out=sq1[:, 1:3, :pw], in0=cp12[:, :, :pw],
                        in1=cp12[:, :, :pw], op=AluOpType.mult)
                    n01 = n01_p.tile([128, 2 * T], F16, tag="n01b",
                                     name=f"n01b{t}")
                    # the solo pair is on the group-boundary critical path
                    # (feeds next group's sqrt): use DVE, not slow GpSimd
                    eng = nc.vector if len(pr) == 1 else nc.gpsimd
                    eng.tensor_add(n01[:, :pw], sq1[:, 1, :pw],
                                   sq1[:, 2, :pw])
                    if ti == 1:
                        N2B[g] = n2b_p.tile([128, GT], F16, tag="n2b",
                                            name=f"n2b_g{g}")
                    eng.tensor_add(N2B[g][:, a0: a0 + pw],
                                   n01[:, :pw], sq1[:, 0, :pw])

            def stage_z(t):
                """z matmuls for a tile whose silu-b ran last k-step.
                They write into the (dead) ph1b bank of the same tile:
                saves an H-pool allocation per tile."""
                st = S[t]
                pzt = st["ph1b"][:, 0:4]
                for j in range(4):
                    nc.tensor.matmul(
                        pzt[:, j: j + 1],
                        st["g2b"][:, j * 128: (j + 1) * 128],
                        weff[:],
                        start=True, stop=True, skip_group_check=True,
                    )
                S[t]["pz"] = pzt
                del S[t]["g2b"]

            def stage_zt(t):
                """zt evac (ACT Identity+bias; in every table set)."""
                st = S[t]
                zt = zz_p.tile([128, 4], F16, tag="zt", name=f"zt{t}")
                nc.scalar.activation(zt[:], st["pz"], AF.Identity,
                                     bias=bias["beff"][:])
                S[t]["zt"] = zt
                del S[t]["pz"]
                del S[t]["ph1b"]

            def pool_burst(tcq, gidx):
                """28 pooling matmuls into one [1,B] H-slot, then one DVE
                add into the SBUF ping-pong accumulator."""
                yp = psH.tile([1, B], F, tag="h", name=f"yp{gidx}")
                n = 4 * len(tcq)
                i = 0
                for t in tcq:
                    st = S[t]
                    for j in range(4):
                        nc.tensor.matmul(
                            yp[:],
                            st["zt"][:, j: j + 1],
                            st["bt"][:, j, :],
                            start=(i == 0), stop=(i == n - 1),
                            skip_group_check=True,
                        )
                        i += 1
                    del S[t]["zt"]
                    del S[t]["bt"]
                    del S[t]
                nc.vector.tensor_add(ysb[gidx % 2][:], ysb[(gidx + 1) % 2][:],
                                     yp[:])

            def dma_group(q):
                a0 = q * GT
                vg = vin_p.tile([128, 3, GT], F8, tag="vg", name=f"vg{q}")
                nc.sync.dma_start(out=vg[:], in_=vT[:, :, a0: a0 + GT])
                sg_in = sin_p.tile([128, GT], F16, tag="sg_in", name=f"sg{q}")
                nc.sync.dma_start(out=sg_in[:], in_=sT[:, a0: a0 + GT])
                for t in range(q * G, (q + 1) * G):
                    S[t] = {"vg": vg, "sg_in": sg_in}

            # ---- main loop: one iteration per group, software-pipelined.
            dma_group(0)
            dma_group(1)
            for t in range(G):
                stage_pre(t)
            for g in range(NG + 1):
                tb = list(range(g * G, (g + 1) * G)) if g < NG else []
                tcq = list(range((g - 1) * G, g * G)) if g > 0 else []

                for t in tcq:
                    bt = bin_p.tile([128, 4, B], F8, tag="bt", name=f"bt{t}")
                    nc.sync.dma_start(
                        out=bt[:],
                        in_=bT[t * T: (t + 1) * T, :].rearrange(
                            "(j p) b -> p j b", p=128
                        ),
                    )
                    S[t]["bt"] = bt

                # ---- sqrt phase (one table load) ----
                # Sqrt splits: bulk (pairs 0-2, sums ready early) first,
                # solo-pair tails (boundary-chain data) last.
                norm0g = norm1g = None
                if tb:
                    norm0g = nrm_p.tile([128, GT], F16, tag="nrm0",
                                        name=f"nrm0_g{g}")
                    _pin(nc.scalar.activation(norm0g[:, : 6 * T],
                                              N2A[g][:, : 6 * T], AF.Sqrt))
                if tcq:
                    norm1g = nrm_p.tile([128, GT], F16, tag="nrm1",
                                        name=f"nrm1_g{g}")
                    _pin(nc.scalar.activation(norm1g[:, : 6 * T],
                                              N2B[g - 1][:, : 6 * T],
                                              AF.Sqrt))
                if tb:
                    _pin(nc.scalar.activation(norm0g[:, 6 * T:],
                                              N2A[g][:, 6 * T:], AF.Sqrt))
                if tcq:
                    _pin(nc.scalar.activation(norm1g[:, 6 * T:],
                                              N2B[g - 1][:, 6 * T:],
                                              AF.Sqrt))
                _end_phase()

                # ---- modulo-scheduled body: each k-step emits stages of
                # DIFFERENT tiles, offset so every op's inputs were
                # produced >= 1 step earlier (in-order engine queues never
                # sit on a not-yet-ready instruction). Only the silus are
                # table-phase-pinned.
                tnx = []
                if g + 2 < NG:
                    dma_group(g + 2)
                if g + 1 < NG:
                    tnx = list(range((g + 1) * G, (g + 2) * G))
                for k in range(G + 1):
                    if 1 <= k and k - 1 < len(tcq):
                        stage_z(tcq[k - 1])        # needs silu-b from k-1
                        stage_zt(tcq[k - 1])       # frees the H bank early
                    if k < len(tb):
                        t = tb[k]
                        g_, ti = divmod(t, G)
                        stt = S[t]["sg_in"][:, ti * T: (ti + 1) * T]
                        ph = psH.tile([128, T], F, tag="h", name=f"ph1a{t}")
                        nc.tensor.matmul(ph[:], w["wh1s_0"][:], stt,
                                         start=True, stop=False,
                                         skip_group_check=True)
                        nc.tensor.matmul(ph[:], w["wh1n_0"][:],
                                         norm0g[:, k * T: (k + 1) * T],
                                         start=False, stop=True,
                                         skip_group_check=True)
                        g2a = g2a_p.tile([128, T], F16, tag="g2a",
                                         name=f"g2a{t}")
                        _pin(nc.scalar.activation(g2a[:], ph[:], AF.Silu,
                                                  bias=bias["b1_0"][:]))
                        S[t]["g2a"] = g2a
                        S[t]["ph1a"] = ph
                    if k < len(tcq):
                        t = tcq[k]
                        ph = psH.tile([128, T], F, tag="h", name=f"ph1b{t}")
                        nc.tensor.matmul(ph[:], w["wh1sg_1"][:], S[t]["g2a"][:],
                                         start=True, stop=False,
                                         skip_group_check=True)
                        nc.tensor.matmul(ph[:], w["wh1n_1"][:],
                                         norm1g[:, k * T: (k + 1) * T],
                                         start=False, stop=True,
                                         skip_group_check=True)
                        g2b = g2b_p.tile([128, T], F16, tag="g2b",
                                         name=f"g2b{t}")
                        _pin(nc.scalar.activation(g2b[:], ph[:], AF.Silu,
                                                  bias=bias["b1_1"][:]))
                        S[t]["g2b"] = g2b
                        S[t]["ph1b"] = ph
                    if 1 <= k and k - 1 < len(tb):
                        stage_tail2(tb[k - 1])     # needs vnew from k-1
                    if k < len(tb):
                        stage_tail1(tb[k])         # needs silu-a from this k
                    if k < len(tnx):
                        stage_pre(tnx[k])
                _end_phase()
                if tcq:
                    pool_burst(tcq, g)

            nc.sync.dma_start(out=y[:], in_=ysb[NG % 2][:])

    _legalize_waits(nc)
    return nc


def _prep_weights(inputs):
    f32 = lambda a: np.asarray(a, np.float32)
    u0_w, v0_w = f32(inputs["u0_w"]), f32(inputs["v0_w"])
    a0_w1, a0_b1 = f32(inputs["a0_w1"]), f32(inputs["a0_b1"])
    a0_w2, a0_b2 = f32(inputs["a0_w2"]), f32(inputs["a0_b2"])
    v1_w = f32(inputs["v1_w"])
    a1_w1, a1_b1 = f32(inputs["a1_w1"]), f32(inputs["a1_b1"])
    a1_w2, a1_b2 = f32(inputs["a1_w2"]), f32(inputs["a1_b2"])
    out_w, out_b = f32(inputs["out_w"]), f32(inputs["out_b"])

    f16c = lambda a: np.ascontiguousarray(a, np.float16)
    f32c = lambda a: np.ascontiguousarray(a.reshape(NF, 1), np.float32)

    # composed block-1 s-path: h1b_sg = (W1s @ Wsg) @ g2a ; bias folded
    W1s = a1_w1[:, :NF]
    Wsg = a0_w2[:NF, :]
    bsg = a0_b2[:NF]
    Wc = W1s @ Wsg
    b1_1_eff = a1_b1 + W1s @ bsg

    return {
        "wv1_0": f16c(u0_w.T),
        "wv2_0": f16c(v0_w.T),
        "wh1s_0": f16c(a0_w1.T[:NF]),
        "wh1n_0": f16c(a0_w1.T[NF:]),
        "wss_0": f16c(a0_w2[NF:].T),
        "wv2_1": f16c(v1_w.T),
        "wh1sg_1": f16c(Wc.T),
        "wh1n_1": f16c(a1_w1.T[NF:]),
        "weff": f16c((out_w[0] @ a1_w2[:NF]).reshape(NF, 1)),
        "b1_0": f32c(np.broadcast_to(a0_b1, (NF,)).copy()),
        "bss_0": f32c(a0_b2[NF:].copy()),
        "b1_1": f32c(b1_1_eff),
        "beff": np.full((NF, 1), float(out_w[0] @ a1_b2[:NF] + out_b[0]),
                        np.float32),
        "eps": np.full((NF, 1), EPS, np.float32),
    }


def kernel(**inputs):
    _install_compat_patches()
    if os.environ.get("BASS_TRACE"):
        _maybe_install_trace_shim()
    from concourse.bass_utils import run_bass_kernel_spmd

    s = np.asarray(inputs["s"], np.float32)
    v = np.asarray(inputs["v"], np.float32)
    batch = np.asarray(inputs["batch"], np.float32)
    weights = _prep_weights(inputs)

    v0 = v[0]            # (NA, 3, NF)
    s0 = s[0]            # (NA, NF)
    bm = batch[:, :, 0]  # (B, NA)

    in_maps = []
    for c in range(NCORES):
        sl = slice(c * SH, (c + 1) * SH)
        import ml_dtypes
        vt8 = np.zeros((NF, 3, NAs), ml_dtypes.float8_e4m3)
        vt8[:, :, :SH] = v0[sl].transpose(2, 1, 0).astype(ml_dtypes.float8_e4m3)
        vt = vt8.view(np.uint8)
        st = np.zeros((NF, NAs), np.float16)
        st[:, :SH] = s0[sl].T
        bt8 = np.zeros((NAs, B), ml_dtypes.float8_e4m3)
        bt8[:SH] = bm[:, sl].T.astype(ml_dtypes.float8_e4m3)
        bt = bt8.view(np.uint8)
        in_maps.append({"vT": vt, "sT": st, "bT": bt, **weights})

    key = "prog"
    if key not in _prog_cache:
        _prog_cache[key] = _build_program()
    nc = _prog_cache[key]

    res = run_bass_kernel_spmd(nc, in_maps, list(range(NCORES)))
    if res.exec_time_ns is not None:
        print(f"HW exec time: {res.exec_time_ns} ns")
    kernel._last_result = res

    ysum = np.zeros((B,), np.float64)
    for c in range(NCORES):
        ysum += res.results[c]["y"].reshape(B).astype(np.float64)
    return ysum.astype(np.float32).reshape(B, 1)
